# revision 15
# baseline (speedup 1.0000x reference)
"""MultiHeadAttention Trainium2 kernel (8-core sharded).

Reference computation (per batch b):
  qh = einsum('sd,hdk->hsk', q[b], Wq) + bq   (same k, v)
  scores = qh @ kh^T / sqrt(64); weights = softmax(scores)
  attn = weights @ vh; out = concat_heads(attn) @ Wo + bo
Returns (out [B,S,D], weights [B,H,S,S]).

Sharding: core c handles batch b = c//4 and heads [4g, 4g+4), g = c%4.
Each core computes its 4 heads' weights plus a partial output projection
(contracting only its heads' slice of Wo); the host sums the 4 partials
per batch and adds the bias terms (bv folds into a constant row because
softmax rows sum to 1: attn = attn_nobv + bv, so out gains bv_flat@Wo).

Device pipeline per core (engines in parentheses):
  1. q/k/v tiles -> PE transpose -> qT chunks (SBUF), projections on PE:
     qhT/khT [dk-on-partitions, s] fp32r, vh [s-on-partitions, dv] bf16
     with an appended ones column (gives softmax denominators for free
     in the attention matmul).
  2. Per head: scoresT chunks (PE, fp32r) -> exp (ACT) -> expT bf16;
     scores chunks (PE) -> exp+rowsum (ACT) -> normalize (DVE, 1/rowsum
     per-partition) -> weights DMA out (fp32).
  3. attn: attnT_un = vh_aug^T @ expT (PE, bf16, rowsum in row 64),
     normalized along q via a DRAM-bounce broadcast of 1/rowsum (DVE).
  4. output projection (PE, bf16) -> partial out DMA (fp32).
"""

import numpy as np
import ml_dtypes

B, S, DIM, H, DK, DV = 2, 2048, 1024, 16, 64, 64
HPC = 4                     # heads per core
NCORES = 8
SCALE = 1.0 / np.sqrt(DV)   # folded into Wq/bq on host
NT = S // 128               # 16 s-tiles
NJ = DIM // 128             # 8 d-chunks

_cache = {}


def _build():
    import concourse.bass as bass
    import concourse.mybir as mybir
    import concourse.tile as tile
    from concourse import bacc
    from contextlib import ExitStack

    f32 = mybir.dt.float32
    f32r = mybir.dt.float32r
    bf16 = mybir.dt.bfloat16
    ACT_COPY = mybir.ActivationFunctionType.Copy
    ACT_EXP = mybir.ActivationFunctionType.Exp

    nc = bacc.Bacc(None, target_bir_lowering=False)

    q_d = nc.dram_tensor("q", [S, DIM], f32r, kind="ExternalInput")
    k_d = nc.dram_tensor("k", [S, DIM], f32r, kind="ExternalInput")
    v_d = nc.dram_tensor("v", [S, DIM], f32r, kind="ExternalInput")
    wq_d = nc.dram_tensor("wq", [DIM, 256], f32r, kind="ExternalInput")
    wk_d = nc.dram_tensor("wk", [DIM, 256], f32r, kind="ExternalInput")
    wv_d = nc.dram_tensor("wv", [DIM, 256], f32r, kind="ExternalInput")
    bq_d = nc.dram_tensor("bq", [128, 2], f32, kind="ExternalInput")
    bk_d = nc.dram_tensor("bk", [128, 2], f32, kind="ExternalInput")
    wo_d = nc.dram_tensor("wo", [256, DIM], bf16, kind="ExternalInput")
    id_d = nc.dram_tensor("ident", [128, 128], f32r, kind="ExternalInput")
    w_out = nc.dram_tensor("w_out", [HPC, S, S], f32, kind="ExternalOutput")
    o_out = nc.dram_tensor("o_out", [S, DIM], f32, kind="ExternalOutput")

    with tile.TileContext(nc) as tc, ExitStack() as ctx:
        singles = ctx.enter_context(tc.tile_pool(name="singles", bufs=1))
        persist = ctx.enter_context(tc.tile_pool(name="persist", bufs=1))
        qin = ctx.enter_context(tc.tile_pool(name="qin", bufs=1))
        qtp = ctx.enter_context(tc.tile_pool(name="qtp", bufs=1))
        wstp2 = ctx.enter_context(tc.tile_pool(name="wstp2", bufs=2))
        ostp = ctx.enter_context(tc.tile_pool(name="ostp", bufs=2))
        small = ctx.enter_context(tc.tile_pool(name="small", bufs=2))
        stage = ctx.enter_context(tc.tile_pool(name="stage", bufs=2))
        psum = ctx.enter_context(tc.tile_pool(name="ps", bufs=2, space="PSUM"))
        dram = ctx.enter_context(tc.tile_pool(name="drs", bufs=2, space="DRAM"))

        # ---- constants ----
        ident = singles.tile([128, 128], f32r, tag="ident")
        nc.sync.dma_start(out=ident, in_=id_d[:, :])
        wq_sb = singles.tile([128, NJ, 256], f32r, tag="wq")
        wk_sb = singles.tile([128, NJ, 256], f32r, tag="wk")
        wv_sb = singles.tile([128, NJ, 256], f32r, tag="wv")
        nc.sync.dma_start(out=wq_sb, in_=wq_d.rearrange("(j p) c -> p j c", p=128))
        nc.sync.dma_start(out=wk_sb, in_=wk_d.rearrange("(j p) c -> p j c", p=128))
        nc.sync.dma_start(out=wv_sb, in_=wv_d.rearrange("(j p) c -> p j c", p=128))
        wo_sb = singles.tile([128, 2, DIM], bf16, tag="wo")
        nc.sync.dma_start(out=wo_sb, in_=wo_d.rearrange("(m p) c -> p m c", p=128))
        bq_sb = singles.tile([128, 2], f32, tag="bq")
        bk_sb = singles.tile([128, 2], f32, tag="bk")
        nc.sync.dma_start(out=bq_sb, in_=bq_d[:, :])
        nc.sync.dma_start(out=bk_sb, in_=bk_d[:, :])

        # ---- persistent activations ----
        qhT = persist.tile([128, 2, S], f32r, tag="qhT")   # [(h%2)*64+dk, h//2, s]
        khT = persist.tile([128, 2, S], f32r, tag="khT")
        vha = persist.tile([128, NT, HPC, 65], bf16, tag="vha")  # [k%128, kt, h, dv|1]
        expT = persist.tile([128, NT, S], bf16, tag="expT")      # per-head reuse
        attnT = persist.tile([128, 2, S], bf16, tag="attnT")     # [(h%2)*64+dv, h//2, q]
        nc.vector.memset(vha[:, :, :, 64:65], 1.0)

        # ---- phase 1: load, transpose, project (512-row slabs) ----
        def load_transpose_project(src_d, which, sp):
            """One 512-row slab of q/k/v: load, build xT chunks, project."""
            xin = qin.tile([128, 4, DIM], f32r, tag="xin")
            for st4 in range(4):
                s0 = sp * 512 + st4 * 128
                nc.sync.dma_start(out=xin[:, st4, :], in_=src_d[s0:s0 + 128, :])
            xt = qtp.tile([128, NJ, 512], f32r, tag="xt")  # [d%128, j, s-in-slab]
            for half in range(2):
                pst = psum.tile([128, 2048], f32r, tag="big")
                for st2 in range(2):
                    st4 = half * 2 + st2
                    for j in range(NJ):
                        nc.tensor.transpose(
                            pst[:, j * 256 + st2 * 128:j * 256 + st2 * 128 + 128],
                            xin[:, st4, j * 128:(j + 1) * 128], ident)
                nc.vector.tensor_copy(
                    out=xt[:, :, half * 256:(half + 1) * 256],
                    in_=pst.rearrange("p (j s) -> p j s", j=NJ))
            if which == "q" or which == "k":
                w_sb = wq_sb if which == "q" else wk_sb
                b_sb = bq_sb if which == "q" else bk_sb
                dstT = qhT if which == "q" else khT
                for m in range(2):
                    pp = psum.tile([128, 512], f32, tag="big")
                    for j in range(NJ):
                        nc.tensor.matmul(pp, w_sb[:, j, m * 128:(m + 1) * 128],
                                         xt[:, j, :],
                                         start=(j == 0), stop=(j == NJ - 1))
                    nc.vector.tensor_scalar_add(
                        dstT[:, m, sp * 512:(sp + 1) * 512], pp, b_sb[:, m:m + 1])
            else:
                for st4 in range(4):
                    pp = psum.tile([128, 256], f32, tag="big")
                    for j in range(NJ):
                        nc.tensor.matmul(pp, xt[:, j, st4 * 128:(st4 + 1) * 128],
                                         wv_sb[:, j, :],
                                         start=(j == 0), stop=(j == NJ - 1))
                    kt = sp * 4 + st4
                    nc.vector.tensor_copy(
                        out=vha[:, kt, :, 0:64],
                        in_=pp.rearrange("p (h d) -> p h d", h=HPC))

        for sp in range(4):
            load_transpose_project(q_d, "q", sp)
            load_transpose_project(k_d, "k", sp)
            load_transpose_project(v_d, "v", sp)

        # bf16 copies of qhT/khT for the attention-side scores (reuses the
        # qtp slot, which is free after phase 1)
        qk_bf = qtp.tile([128, 2, 2, S], bf16, tag="xt")  # [part, q|k, hp, s]
        nc.vector.tensor_copy(out=qk_bf[:, 0], in_=qhT.bitcast(f32))
        nc.vector.tensor_copy(out=qk_bf[:, 1], in_=khT.bitcast(f32))

        # ---- phase 2: per-head attention ----
        for h in range(HPC):
            p0 = (h % 2) * 64
            hp = h // 2
            # (a) scoresT -> expT (bf16)
            for t in range(NT):
                psA = psum.tile([128, 2048], f32, tag="big")
                for qc in range(4):
                    nc.tensor.matmul(
                        psA[:, qc * 512:(qc + 1) * 512],
                        qk_bf[p0:p0 + 64, 1, hp, t * 128:(t + 1) * 128],
                        qk_bf[p0:p0 + 64, 0, hp, qc * 512:(qc + 1) * 512],
                        start=True, stop=True)
                nc.scalar.activation(out=expT[:, t, :], in_=psA, func=ACT_EXP)

            # (b) scores -> exp+rowsum -> normalize -> weights DMA
            # interleaved with (c) attention groups
            def b_chunk(t):
                psB = psum.tile([128, 2048], f32, tag="big")
                for kc in range(4):
                    nc.tensor.matmul(
                        psB[:, kc * 512:(kc + 1) * 512],
                        qhT[p0:p0 + 64, hp, t * 128:(t + 1) * 128],
                        khT[p0:p0 + 64, hp, kc * 512:(kc + 1) * 512],
                        start=True, stop=True)
                wraw = wstp2.tile([128, 2048], f32, tag="wnorm")
                rs = small.tile([128, 1], f32, tag="rs")
                nc.scalar.activation(out=wraw, in_=psB, func=ACT_EXP, accum_out=rs)
                wnorm = wstp2.tile([128, 2048], f32, tag="wnorm")
                nc.gpsimd.normalize_recip(wnorm, wraw, rs)
                nc.sync.dma_start(out=w_out[h, t * 128:(t + 1) * 128, :], in_=wnorm)

            def attn_group(qt):
                psAt = psum.tile([65, 512], f32, tag="big")
                for kt in range(NT):
                    nc.tensor.matmul(
                        psAt, vha[:, kt, h, :],
                        expT[:, kt, qt * 512:(qt + 1) * 512],
                        start=(kt == 0), stop=(kt == NT - 1))
                rf = small.tile([1, 512], bf16, tag="rf")
                with nc.allow_low_precision(reason="softmax denom in bf16 attn path"):
                    nc.vector.reciprocal(out=rf, in_=psAt[64:65, :])
                stg = stage.tile([64, 512], bf16, tag="stg")
                nc.vector.tensor_copy(out=stg, in_=psAt[0:64, :])
                drs = dram.tile([1, 512], bf16, tag="drs")
                nc.sync.dma_start(out=drs, in_=rf)
                bc = stage.tile([64, 512], bf16, tag="bc")
                nc.gpsimd.dma_start(out=bc, in_=drs.to_broadcast([64, 512]))
                nc.vector.tensor_mul(
                    attnT[p0:p0 + 64, hp, qt * 512:(qt + 1) * 512],
                    stg, bc)

            for t in range(NT):
                b_chunk(t)
                if t % 4 == 3:
                    attn_group(t // 4)

        # ---- phase 3: output projection (partial, no bias) ----
        for st in range(NT):
            psO = psum.tile([128, 1024], f32, tag="big")
            for oc in range(2):
                for m in range(2):
                    nc.tensor.matmul(
                        psO[:, oc * 512:(oc + 1) * 512],
                        attnT[:, m, st * 128:(st + 1) * 128],
                        wo_sb[:, m, oc * 512:(oc + 1) * 512],
                        start=(m == 0), stop=(m == 1))
            ost = ostp.tile([128, DIM], f32, tag="ost")
            nc.scalar.copy(out=ost, in_=psO)
            nc.sync.dma_start(out=o_out[st * 128:(st + 1) * 128, :], in_=ost)

    nc.compile()
    return nc


def _prep_core_inputs(c, q, k, v, Wq, bq, Wk, bk, Wv, Wo, ident):
    b, g = divmod(c, HPC)
    hs = g * HPC
    sl = slice(hs, hs + HPC)
    return {
        "q": np.ascontiguousarray(q[b]),
        "k": np.ascontiguousarray(k[b]),
        "v": np.ascontiguousarray(v[b]),
        "wq": np.ascontiguousarray(
            (Wq[sl] * SCALE).transpose(1, 0, 2).reshape(DIM, HPC * DK).astype(np.float32)),
        "wk": np.ascontiguousarray(
            Wk[sl].transpose(1, 0, 2).reshape(DIM, HPC * DK).astype(np.float32)),
        "wv": np.ascontiguousarray(
            Wv[sl].transpose(1, 0, 2).reshape(DIM, HPC * DV).astype(np.float32)),
        "bq": np.ascontiguousarray(
            (bq[sl] * SCALE).reshape(HPC * DK).reshape(2, 128).T.astype(np.float32)),
        "bk": np.ascontiguousarray(
            bk[sl].reshape(HPC * DK).reshape(2, 128).T.astype(np.float32)),
        "wo": np.ascontiguousarray(
            Wo[g * 256:(g + 1) * 256].astype(ml_dtypes.bfloat16)),
        "ident": ident,
    }


def kernel(q, k, v, Wq, bq, Wk, bk, Wv, bv, Wo, bo):
    from concourse.bass_utils import run_bass_kernel_spmd

    q, k, v = (np.asarray(x, np.float32) for x in (q, k, v))
    Wq, bq, Wk, bk, Wv, bv, Wo, bo = (
        np.asarray(x, np.float32) for x in (Wq, bq, Wk, bk, Wv, bv, Wo, bo))

    if "nc" not in _cache:
        _cache["nc"] = _build()
    nc = _cache["nc"]

    ident = np.eye(128, dtype=np.float32)
    in_maps = [_prep_core_inputs(c, q, k, v, Wq, bq, Wk, bk, Wv, Wo, ident)
               for c in range(NCORES)]
    res = run_bass_kernel_spmd(nc, in_maps, core_ids=list(range(NCORES))).results

    weights = np.empty((B, H, S, S), np.float32)
    out = np.empty((B, S, DIM), np.float32)
    extra = (bv.reshape(DIM) @ Wo + bo).astype(np.float32)  # bv folds via softmax sum=1
    for b in range(B):
        acc = None
        for g in range(HPC):
            r = res[b * HPC + g]
            weights[b, g * HPC:(g + 1) * HPC] = r["w_out"]
            acc = r["o_out"] if acc is None else acc + r["o_out"]
        out[b] = acc + extra
    return out, weights


# revision 19
# speedup vs baseline: 1.2550x; 1.2550x over previous
"""MultiHeadAttention Trainium2 kernel (8-core sharded).

Reference computation (per batch b):
  qh = einsum('sd,hdk->hsk', q[b], Wq) + bq   (same k, v)
  scores = qh @ kh^T / sqrt(64); weights = softmax(scores)
  attn = weights @ vh; out = concat_heads(attn) @ Wo + bo
Returns (out [B,S,D], weights [B,H,S,S]).

Sharding: core c handles batch b = c//4 and heads [4g, 4g+4), g = c%4.
Each core computes its 4 heads' weights plus a partial output projection
(contracting only its heads' slice of Wo); the host sums the 4 partials
per batch and adds the bias terms (bv folds into a constant row because
softmax rows sum to 1: attn = attn_nobv + bv, so out gains bv_flat@Wo).
The host ships q/k/v pre-transposed ([D, S]) so the device needs no PE
transposes for the projections.

Device pipeline per core:
  1. projections on PE from qT/kT/vT chunks (fp32r):
     qhT/khT [dk-on-partitions, s] fp32r (+ bf16 copies), vh bf16.
  2. Per head:
     (a) scoresT chunks (PE, bf16) -> exp (ACT) -> expT bf16
     (b) scores chunks (PE, fp32r) -> exp+rowsum (ACT) -> normalize
         (GpSimd normalize_recip; denominators become reciprocals
         in-place) -> weights DMA (fp32, accurate path)
     (c) attn: attnT_un = vh^T @ expT (PE, bf16); normalized along q
         using the (b) reciprocals: PE-mini-transpose -> DRAM bounce ->
         partition-broadcast load (GpSimd, cast bf16) -> DVE multiply.
  3. output projection (PE, bf16) -> partial out DMA (fp32).
"""

import numpy as np
import ml_dtypes

B, S, DIM, H, DK, DV = 2, 2048, 1024, 16, 64, 64
HPC = 4                     # heads per core
NCORES = 8
SCALE = 1.0 / np.sqrt(DV)   # folded into Wq/bq on host
NT = S // 128               # 16 s-tiles
NJ = DIM // 128             # 8 d-chunks

_cache = {}


def _build():
    import concourse.bass as bass
    import concourse.mybir as mybir
    import concourse.tile as tile
    from concourse import bacc
    from contextlib import ExitStack

    f32 = mybir.dt.float32
    f32r = mybir.dt.float32r
    bf16 = mybir.dt.bfloat16
    ACT_EXP = mybir.ActivationFunctionType.Exp

    nc = bacc.Bacc(None, target_bir_lowering=False)

    qt_d = nc.dram_tensor("qT", [DIM, S], f32r, kind="ExternalInput")
    kt_d = nc.dram_tensor("kT", [DIM, S], f32r, kind="ExternalInput")
    vt_d = nc.dram_tensor("vT", [DIM, S], f32r, kind="ExternalInput")
    wq_d = nc.dram_tensor("wq", [DIM, 256], f32r, kind="ExternalInput")
    wk_d = nc.dram_tensor("wk", [DIM, 256], f32r, kind="ExternalInput")
    wv_d = nc.dram_tensor("wv", [DIM, 256], f32r, kind="ExternalInput")
    bq_d = nc.dram_tensor("bq", [128, 2], f32, kind="ExternalInput")
    bk_d = nc.dram_tensor("bk", [128, 2], f32, kind="ExternalInput")
    wo_d = nc.dram_tensor("wo", [256, DIM], bf16, kind="ExternalInput")
    id_d = nc.dram_tensor("ident", [128, 128], f32, kind="ExternalInput")
    w_out = nc.dram_tensor("w_out", [HPC, S, S], f32, kind="ExternalOutput")
    o_out = nc.dram_tensor("o_out", [S, DIM], f32, kind="ExternalOutput")

    with tile.TileContext(nc) as tc, ExitStack() as ctx:
        singles = ctx.enter_context(tc.tile_pool(name="singles", bufs=1))
        persist = ctx.enter_context(tc.tile_pool(name="persist", bufs=1))
        qtp = ctx.enter_context(tc.tile_pool(name="qtp", bufs=1))
        wstp2 = ctx.enter_context(tc.tile_pool(name="wstp2", bufs=4))
        ostp = ctx.enter_context(tc.tile_pool(name="ostp", bufs=2))
        small = ctx.enter_context(tc.tile_pool(name="small", bufs=2))
        stage = ctx.enter_context(tc.tile_pool(name="stage", bufs=4))
        bcp = ctx.enter_context(tc.tile_pool(name="bcp", bufs=1))
        psum = ctx.enter_context(tc.tile_pool(name="ps", bufs=2, space="PSUM"))
        dram = ctx.enter_context(tc.tile_pool(name="drs", bufs=2, space="DRAM"))

        # ---- constants ----
        ident = singles.tile([128, 128], f32, tag="ident")
        nc.sync.dma_start(out=ident, in_=id_d[:, :])
        wq_sb = singles.tile([128, NJ, 256], f32r, tag="wq")
        wk_sb = singles.tile([128, NJ, 256], f32r, tag="wk")
        wv_sb = singles.tile([128, NJ, 256], f32r, tag="wv")
        nc.sync.dma_start(out=wq_sb, in_=wq_d.rearrange("(j p) c -> p j c", p=128))
        nc.sync.dma_start(out=wk_sb, in_=wk_d.rearrange("(j p) c -> p j c", p=128))
        nc.sync.dma_start(out=wv_sb, in_=wv_d.rearrange("(j p) c -> p j c", p=128))
        wo_sb = singles.tile([128, 2, DIM], bf16, tag="wo")
        nc.sync.dma_start(out=wo_sb, in_=wo_d.rearrange("(m p) c -> p m c", p=128))
        bq_sb = singles.tile([128, 2], f32, tag="bq")
        bk_sb = singles.tile([128, 2], f32, tag="bk")
        nc.sync.dma_start(out=bq_sb, in_=bq_d[:, :])
        nc.sync.dma_start(out=bk_sb, in_=bk_d[:, :])

        # ---- persistent activations ----
        qhT = persist.tile([128, 2, S], f32r, tag="qhT")   # [(h%2)*64+dk, h//2, s]
        khT = persist.tile([128, 2, S], f32r, tag="khT")
        vha = persist.tile([128, NT, HPC, 64], bf16, tag="vha")  # [k%128, kt, h, dv]
        expT = persist.tile([128, NT, S], bf16, tag="expT")      # per-head reuse
        attnT = persist.tile([128, 2, S], bf16, tag="attnT")     # [(h%2)*64+dv, h//2, q]

        # ---- phase 1: load transposed chunks, project ----
        def load_project(src_d, which, sc):
            """One 512-col slab of qT/kT/vT: load, project."""
            xt = qtp.tile([128, NJ, 512], f32r, tag="xt")  # [d%128, j, s-in-slab]
            nc.sync.dma_start(
                out=xt,
                in_=src_d.rearrange("(j p) s -> p j s", p=128)[
                    :, :, sc * 512:(sc + 1) * 512])
            if which == "q" or which == "k":
                w_sb = wq_sb if which == "q" else wk_sb
                b_sb = bq_sb if which == "q" else bk_sb
                dstT = qhT if which == "q" else khT
                for m in range(2):
                    pp = psum.tile([128, 512], f32, tag="big")
                    for j in range(NJ):
                        nc.tensor.matmul(pp, w_sb[:, j, m * 128:(m + 1) * 128],
                                         xt[:, j, :],
                                         start=(j == 0), stop=(j == NJ - 1))
                    nc.vector.tensor_scalar_add(
                        dstT[:, m, sc * 512:(sc + 1) * 512], pp, b_sb[:, m:m + 1])
            else:
                for st4 in range(4):
                    pp = psum.tile([128, 256], f32, tag="big")
                    for j in range(NJ):
                        nc.tensor.matmul(pp, xt[:, j, st4 * 128:(st4 + 1) * 128],
                                         wv_sb[:, j, :],
                                         start=(j == 0), stop=(j == NJ - 1))
                    kt = sc * 4 + st4
                    nc.vector.tensor_copy(
                        out=vha[:, kt, :, :],
                        in_=pp.rearrange("p (h d) -> p h d", h=HPC))

        for sc in range(4):
            load_project(qt_d, "q", sc)
            load_project(kt_d, "k", sc)
            load_project(vt_d, "v", sc)

        # bf16 copies of qhT/khT for the attention-side scores (reuses the
        # qtp slot, which is free after phase 1)
        qk_bf = qtp.tile([128, 2, 2, S], bf16, tag="xt")  # [part, q|k, hp, s]
        nc.vector.tensor_copy(out=qk_bf[:, 0], in_=qhT.bitcast(f32))
        nc.vector.tensor_copy(out=qk_bf[:, 1], in_=khT.bitcast(f32))

        # ---- phase 2: per-head attention ----
        for h in range(HPC):
            p0 = (h % 2) * 64
            hp = h // 2
            # (a) scoresT -> expT (bf16)
            for t in range(NT):
                psA = psum.tile([128, 2048], f32, tag="big")
                for qc in range(4):
                    nc.tensor.matmul(
                        psA[:, qc * 512:(qc + 1) * 512],
                        qk_bf[p0:p0 + 64, 1, hp, t * 128:(t + 1) * 128],
                        qk_bf[p0:p0 + 64, 0, hp, qc * 512:(qc + 1) * 512],
                        start=True, stop=True)
                nc.scalar.activation(out=expT[:, t, :], in_=psA, func=ACT_EXP)

            # per-head softmax denominators; normalize_recip overwrites each
            # column with its reciprocal, so after the (b) loop rsh holds
            # 1/rowsum for all 16 q-blocks
            rsh = small.tile([128, NT], f32, tag="rsh")

            # (b) scores -> exp+rowsum -> normalize -> weights DMA
            def b_chunk(t):
                psB = psum.tile([128, 2048], f32, tag="big")
                for kc in range(4):
                    nc.tensor.matmul(
                        psB[:, kc * 512:(kc + 1) * 512],
                        qhT[p0:p0 + 64, hp, t * 128:(t + 1) * 128],
                        khT[p0:p0 + 64, hp, kc * 512:(kc + 1) * 512],
                        start=True, stop=True)
                wraw = wstp2.tile([128, 2048], f32, tag="wnorm")
                nc.scalar.activation(out=wraw, in_=psB, func=ACT_EXP,
                                     accum_out=rsh[:, t:t + 1])
                wnorm = wstp2.tile([128, 2048], f32, tag="wnorm")
                nc.gpsimd.normalize_recip(wnorm, wraw, rsh[:, t:t + 1])
                nc.sync.dma_start(out=w_out[h, t * 128:(t + 1) * 128, :], in_=wnorm)

            # (c) attention matmuls (normalization happens after (b))
            def attn_group(qt):
                psAt = psum.tile([64, 512], f32, tag="big")
                for kt in range(NT):
                    nc.tensor.matmul(
                        psAt, vha[:, kt, h, :],
                        expT[:, kt, qt * 512:(qt + 1) * 512],
                        start=(kt == 0), stop=(kt == NT - 1))
                stg = stage.tile([64, 512], bf16, tag="stg")
                nc.vector.tensor_copy(out=stg, in_=psAt)
                return stg

            stgs = []
            for t in range(NT):
                b_chunk(t)
                if t % 4 == 3:
                    stgs.append(attn_group(t // 4))

            # broadcast 1/rowsum along q (free axis): transpose rsh on PE,
            # bounce through DRAM, partition-broadcast load with bf16 cast
            psr = psum.tile([NT, 128], f32, tag="big")
            nc.tensor.transpose(psr, rsh, ident)
            rcT = small.tile([NT, 128], f32, tag="rcT")
            nc.vector.tensor_copy(out=rcT, in_=psr)
            drh = dram.tile([NT, 128], f32, tag="drh")
            nc.sync.dma_start(out=drh, in_=rcT)
            bc = bcp.tile([64, S], bf16, tag="bc")
            nc.gpsimd.dma_start(
                out=bc, in_=drh.rearrange("t (o p) -> o (t p)", o=1).to_broadcast([64, S]))
            for qt in range(4):
                nc.vector.tensor_mul(
                    attnT[p0:p0 + 64, hp, qt * 512:(qt + 1) * 512],
                    stgs[qt], bc[:, qt * 512:(qt + 1) * 512])

        # ---- phase 3: output projection (partial, no bias) ----
        for st in range(NT):
            psO = psum.tile([128, 1024], f32, tag="big")
            for oc in range(2):
                for m in range(2):
                    nc.tensor.matmul(
                        psO[:, oc * 512:(oc + 1) * 512],
                        attnT[:, m, st * 128:(st + 1) * 128],
                        wo_sb[:, m, oc * 512:(oc + 1) * 512],
                        start=(m == 0), stop=(m == 1))
            ost = ostp.tile([128, DIM], f32, tag="ost")
            nc.scalar.copy(out=ost, in_=psO)
            nc.sync.dma_start(out=o_out[st * 128:(st + 1) * 128, :], in_=ost)

    nc.compile()
    return nc


def _prep_core_inputs(c, q, k, v, Wq, bq, Wk, bk, Wv, Wo, ident):
    b, g = divmod(c, HPC)
    hs = g * HPC
    sl = slice(hs, hs + HPC)
    return {
        "qT": np.ascontiguousarray(q[b].T),
        "kT": np.ascontiguousarray(k[b].T),
        "vT": np.ascontiguousarray(v[b].T),
        "wq": np.ascontiguousarray(
            (Wq[sl] * SCALE).transpose(1, 0, 2).reshape(DIM, HPC * DK).astype(np.float32)),
        "wk": np.ascontiguousarray(
            Wk[sl].transpose(1, 0, 2).reshape(DIM, HPC * DK).astype(np.float32)),
        "wv": np.ascontiguousarray(
            Wv[sl].transpose(1, 0, 2).reshape(DIM, HPC * DV).astype(np.float32)),
        "bq": np.ascontiguousarray(
            (bq[sl] * SCALE).reshape(HPC * DK).reshape(2, 128).T.astype(np.float32)),
        "bk": np.ascontiguousarray(
            bk[sl].reshape(HPC * DK).reshape(2, 128).T.astype(np.float32)),
        "wo": np.ascontiguousarray(
            Wo[g * 256:(g + 1) * 256].astype(ml_dtypes.bfloat16)),
        "ident": ident,
    }


def kernel(q, k, v, Wq, bq, Wk, bk, Wv, bv, Wo, bo):
    from concourse.bass_utils import run_bass_kernel_spmd

    q, k, v = (np.asarray(x, np.float32) for x in (q, k, v))
    Wq, bq, Wk, bk, Wv, bv, Wo, bo = (
        np.asarray(x, np.float32) for x in (Wq, bq, Wk, bk, Wv, bv, Wo, bo))

    if "nc" not in _cache:
        _cache["nc"] = _build()
    nc = _cache["nc"]

    ident = np.eye(128, dtype=np.float32)
    in_maps = [_prep_core_inputs(c, q, k, v, Wq, bq, Wk, bk, Wv, Wo, ident)
               for c in range(NCORES)]
    res = run_bass_kernel_spmd(nc, in_maps, core_ids=list(range(NCORES))).results

    weights = np.empty((B, H, S, S), np.float32)
    out = np.empty((B, S, DIM), np.float32)
    extra = (bv.reshape(DIM) @ Wo + bo).astype(np.float32)  # bv folds via softmax sum=1
    for b in range(B):
        acc = None
        for g in range(HPC):
            r = res[b * HPC + g]
            weights[b, g * HPC:(g + 1) * HPC] = r["w_out"]
            acc = r["o_out"] if acc is None else acc + r["o_out"]
        out[b] = acc + extra
    return out, weights


# revision 20
# speedup vs baseline: 1.3914x; 1.1087x over previous
"""MultiHeadAttention Trainium2 kernel (8-core sharded).

Reference computation (per batch b):
  qh = einsum('sd,hdk->hsk', q[b], Wq) + bq   (same k, v)
  scores = qh @ kh^T / sqrt(64); weights = softmax(scores)
  attn = weights @ vh; out = concat_heads(attn) @ Wo + bo
Returns (out [B,S,D], weights [B,H,S,S]).

Sharding: core c handles batch b = c//4 and heads [4g, 4g+4), g = c%4.
Each core computes its 4 heads' weights plus a partial output projection
(contracting only its heads' slice of Wo); the host sums the 4 partials
per batch and adds the bias terms (bv folds into a constant row because
softmax rows sum to 1: attn = attn_nobv + bv, so out gains bv_flat@Wo).
The host ships q/k/v pre-transposed ([D, S]) so the device needs no PE
transposes for the projections.

Device pipeline per core:
  1. projections on PE from qT/kT/vT chunks (fp32r):
     qhT/khT [dk-on-partitions, s] fp32r (+ bf16 copies), vh bf16.
  2. Per head:
     (a) scoresT chunks (PE, bf16) -> exp (ACT) -> expT bf16
     (b) scores chunks (PE, fp32r) -> exp+rowsum (ACT) -> normalize
         (GpSimd normalize_recip; denominators become reciprocals
         in-place) -> weights DMA (fp32, accurate path)
     (c) attn: attnT_un = vh^T @ expT (PE, bf16); normalized along q
         using the (b) reciprocals: PE-mini-transpose -> DRAM bounce ->
         partition-broadcast load (GpSimd, cast bf16) -> DVE multiply.
  3. output projection (PE, bf16) -> partial out DMA (fp32).
"""

import numpy as np
import ml_dtypes

B, S, DIM, H, DK, DV = 2, 2048, 1024, 16, 64, 64
HPC = 4                     # heads per core
NCORES = 8
SCALE = 1.0 / np.sqrt(DV)   # folded into Wq/bq on host
NT = S // 128               # 16 s-tiles
NJ = DIM // 128             # 8 d-chunks

_cache = {}


def _build():
    import concourse.bass as bass
    import concourse.mybir as mybir
    import concourse.tile as tile
    from concourse import bacc
    from contextlib import ExitStack

    f32 = mybir.dt.float32
    f32r = mybir.dt.float32r
    bf16 = mybir.dt.bfloat16
    ACT_EXP = mybir.ActivationFunctionType.Exp

    nc = bacc.Bacc(None, target_bir_lowering=False)

    qt_d = nc.dram_tensor("qT", [DIM, S], bf16, kind="ExternalInput")
    kt_d = nc.dram_tensor("kT", [DIM, S], bf16, kind="ExternalInput")
    vt_d = nc.dram_tensor("vT", [DIM, S], bf16, kind="ExternalInput")
    wq_d = nc.dram_tensor("wq", [DIM, 256], bf16, kind="ExternalInput")
    wk_d = nc.dram_tensor("wk", [DIM, 256], bf16, kind="ExternalInput")
    wv_d = nc.dram_tensor("wv", [DIM, 256], bf16, kind="ExternalInput")
    bq_d = nc.dram_tensor("bq", [128, 2], f32, kind="ExternalInput")
    bk_d = nc.dram_tensor("bk", [128, 2], f32, kind="ExternalInput")
    wo_d = nc.dram_tensor("wo", [256, DIM], bf16, kind="ExternalInput")
    id_d = nc.dram_tensor("ident", [128, 128], f32, kind="ExternalInput")
    w_out = nc.dram_tensor("w_out", [HPC, S, S], f32, kind="ExternalOutput")
    o_out = nc.dram_tensor("o_out", [S, DIM], f32, kind="ExternalOutput")

    with tile.TileContext(nc) as tc, ExitStack() as ctx:
        singles = ctx.enter_context(tc.tile_pool(name="singles", bufs=1))
        persist = ctx.enter_context(tc.tile_pool(name="persist", bufs=1))
        qtp = ctx.enter_context(tc.tile_pool(name="qtp", bufs=1))
        wstp2 = ctx.enter_context(tc.tile_pool(name="wstp2", bufs=4))
        ostp = ctx.enter_context(tc.tile_pool(name="ostp", bufs=2))
        small = ctx.enter_context(tc.tile_pool(name="small", bufs=2))
        stage = ctx.enter_context(tc.tile_pool(name="stage", bufs=4))
        bcp = ctx.enter_context(tc.tile_pool(name="bcp", bufs=1))
        psum = ctx.enter_context(tc.tile_pool(name="ps", bufs=2, space="PSUM"))
        dram = ctx.enter_context(tc.tile_pool(name="drs", bufs=2, space="DRAM"))

        # ---- constants ----
        ident = singles.tile([128, 128], f32, tag="ident")
        nc.sync.dma_start(out=ident, in_=id_d[:, :])
        wq_sb = singles.tile([128, NJ, 256], bf16, tag="wq")
        wk_sb = singles.tile([128, NJ, 256], bf16, tag="wk")
        wv_sb = singles.tile([128, NJ, 256], bf16, tag="wv")
        nc.sync.dma_start(out=wq_sb, in_=wq_d.rearrange("(j p) c -> p j c", p=128))
        nc.sync.dma_start(out=wk_sb, in_=wk_d.rearrange("(j p) c -> p j c", p=128))
        nc.sync.dma_start(out=wv_sb, in_=wv_d.rearrange("(j p) c -> p j c", p=128))
        wo_sb = singles.tile([128, 2, DIM], bf16, tag="wo")
        nc.sync.dma_start(out=wo_sb, in_=wo_d.rearrange("(m p) c -> p m c", p=128))
        bq_sb = singles.tile([128, 2], f32, tag="bq")
        bk_sb = singles.tile([128, 2], f32, tag="bk")
        nc.sync.dma_start(out=bq_sb, in_=bq_d[:, :])
        nc.sync.dma_start(out=bk_sb, in_=bk_d[:, :])

        # ---- persistent activations ----
        qhT = persist.tile([128, 2, S], bf16, tag="qhT")   # [(h%2)*64+dk, h//2, s]
        khT = persist.tile([128, 2, S], bf16, tag="khT")
        vha = persist.tile([128, NT, HPC, 64], bf16, tag="vha")  # [k%128, kt, h, dv]
        expT = persist.tile([128, NT, S], bf16, tag="expT")      # per-head reuse
        attnT = persist.tile([128, 2, S], bf16, tag="attnT")     # [(h%2)*64+dv, h//2, q]

        # ---- phase 1: load transposed chunks, project ----
        def load_project(src_d, which, sc):
            """One 512-col slab of qT/kT/vT: load, project."""
            xt = qtp.tile([128, NJ, 512], bf16, tag="xt")  # [d%128, j, s-in-slab]
            nc.sync.dma_start(
                out=xt,
                in_=src_d.rearrange("(j p) s -> p j s", p=128)[
                    :, :, sc * 512:(sc + 1) * 512])
            if which == "q" or which == "k":
                w_sb = wq_sb if which == "q" else wk_sb
                b_sb = bq_sb if which == "q" else bk_sb
                dstT = qhT if which == "q" else khT
                for m in range(2):
                    pp = psum.tile([128, 512], f32, tag="big")
                    for j in range(NJ):
                        nc.tensor.matmul(pp, w_sb[:, j, m * 128:(m + 1) * 128],
                                         xt[:, j, :],
                                         start=(j == 0), stop=(j == NJ - 1))
                    with nc.allow_low_precision(reason="bf16 activations"):
                        nc.vector.tensor_scalar_add(
                            dstT[:, m, sc * 512:(sc + 1) * 512], pp,
                            b_sb[:, m:m + 1])
            else:
                for st4 in range(4):
                    pp = psum.tile([128, 256], f32, tag="big")
                    for j in range(NJ):
                        nc.tensor.matmul(pp, xt[:, j, st4 * 128:(st4 + 1) * 128],
                                         wv_sb[:, j, :],
                                         start=(j == 0), stop=(j == NJ - 1))
                    kt = sc * 4 + st4
                    nc.vector.tensor_copy(
                        out=vha[:, kt, :, :],
                        in_=pp.rearrange("p (h d) -> p h d", h=HPC))

        for sc in range(4):
            load_project(qt_d, "q", sc)
            load_project(kt_d, "k", sc)
            load_project(vt_d, "v", sc)

        # ---- phase 2: per-head attention ----
        for h in range(HPC):
            p0 = (h % 2) * 64
            hp = h // 2
            # (a) scoresT -> expT (bf16)
            for t in range(NT):
                psA = psum.tile([128, 2048], f32, tag="big")
                for qc in range(4):
                    nc.tensor.matmul(
                        psA[:, qc * 512:(qc + 1) * 512],
                        khT[p0:p0 + 64, hp, t * 128:(t + 1) * 128],
                        qhT[p0:p0 + 64, hp, qc * 512:(qc + 1) * 512],
                        start=True, stop=True)
                nc.scalar.activation(out=expT[:, t, :], in_=psA, func=ACT_EXP)

            # per-head softmax denominators; normalize_recip overwrites each
            # column with its reciprocal, so after the (b) loop rsh holds
            # 1/rowsum for all 16 q-blocks
            rsh = small.tile([128, NT], f32, tag="rsh")

            # (b) scores -> exp+rowsum -> normalize -> weights DMA
            def b_chunk(t):
                psB = psum.tile([128, 2048], f32, tag="big")
                for kc in range(4):
                    nc.tensor.matmul(
                        psB[:, kc * 512:(kc + 1) * 512],
                        qhT[p0:p0 + 64, hp, t * 128:(t + 1) * 128],
                        khT[p0:p0 + 64, hp, kc * 512:(kc + 1) * 512],
                        start=True, stop=True)
                wraw = wstp2.tile([128, 2048], f32, tag="wnorm")
                nc.scalar.activation(out=wraw, in_=psB, func=ACT_EXP,
                                     accum_out=rsh[:, t:t + 1])
                wnorm = wstp2.tile([128, 2048], f32, tag="wnorm")
                nc.gpsimd.normalize_recip(wnorm, wraw, rsh[:, t:t + 1])
                nc.sync.dma_start(out=w_out[h, t * 128:(t + 1) * 128, :], in_=wnorm)

            # (c) attention matmuls (normalization happens after (b))
            def attn_group(qt):
                psAt = psum.tile([64, 512], f32, tag="big")
                for kt in range(NT):
                    nc.tensor.matmul(
                        psAt, vha[:, kt, h, :],
                        expT[:, kt, qt * 512:(qt + 1) * 512],
                        start=(kt == 0), stop=(kt == NT - 1))
                stg = stage.tile([64, 512], bf16, tag="stg")
                nc.vector.tensor_copy(out=stg, in_=psAt)
                return stg

            stgs = []
            for t in range(NT):
                b_chunk(t)
                if t % 4 == 3:
                    stgs.append(attn_group(t // 4))

            # broadcast 1/rowsum along q (free axis): transpose rsh on PE,
            # bounce through DRAM, partition-broadcast load with bf16 cast
            psr = psum.tile([NT, 128], f32, tag="big")
            nc.tensor.transpose(psr, rsh, ident)
            rcT = small.tile([NT, 128], f32, tag="rcT")
            nc.vector.tensor_copy(out=rcT, in_=psr)
            drh = dram.tile([NT, 128], f32, tag="drh")
            nc.sync.dma_start(out=drh, in_=rcT)
            bc = bcp.tile([64, S], bf16, tag="bc")
            nc.gpsimd.dma_start(
                out=bc, in_=drh.rearrange("t (o p) -> o (t p)", o=1).to_broadcast([64, S]))
            for qt in range(4):
                nc.vector.tensor_mul(
                    attnT[p0:p0 + 64, hp, qt * 512:(qt + 1) * 512],
                    stgs[qt], bc[:, qt * 512:(qt + 1) * 512])

        # ---- phase 3: output projection (partial, no bias) ----
        for st in range(NT):
            psO = psum.tile([128, 1024], f32, tag="big")
            for oc in range(2):
                for m in range(2):
                    nc.tensor.matmul(
                        psO[:, oc * 512:(oc + 1) * 512],
                        attnT[:, m, st * 128:(st + 1) * 128],
                        wo_sb[:, m, oc * 512:(oc + 1) * 512],
                        start=(m == 0), stop=(m == 1))
            ost = ostp.tile([128, DIM], f32, tag="ost")
            nc.scalar.copy(out=ost, in_=psO)
            nc.sync.dma_start(out=o_out[st * 128:(st + 1) * 128, :], in_=ost)

    nc.compile()
    return nc


def _prep_core_inputs(c, q, k, v, Wq, bq, Wk, bk, Wv, Wo, ident):
    b, g = divmod(c, HPC)
    hs = g * HPC
    sl = slice(hs, hs + HPC)
    return {
        "qT": np.ascontiguousarray(q[b].T.astype(ml_dtypes.bfloat16)),
        "kT": np.ascontiguousarray(k[b].T.astype(ml_dtypes.bfloat16)),
        "vT": np.ascontiguousarray(v[b].T.astype(ml_dtypes.bfloat16)),
        "wq": np.ascontiguousarray(
            (Wq[sl] * SCALE).transpose(1, 0, 2).reshape(DIM, HPC * DK)
            .astype(ml_dtypes.bfloat16)),
        "wk": np.ascontiguousarray(
            Wk[sl].transpose(1, 0, 2).reshape(DIM, HPC * DK)
            .astype(ml_dtypes.bfloat16)),
        "wv": np.ascontiguousarray(
            Wv[sl].transpose(1, 0, 2).reshape(DIM, HPC * DV)
            .astype(ml_dtypes.bfloat16)),
        "bq": np.ascontiguousarray(
            (bq[sl] * SCALE).reshape(HPC * DK).reshape(2, 128).T.astype(np.float32)),
        "bk": np.ascontiguousarray(
            bk[sl].reshape(HPC * DK).reshape(2, 128).T.astype(np.float32)),
        "wo": np.ascontiguousarray(
            Wo[g * 256:(g + 1) * 256].astype(ml_dtypes.bfloat16)),
        "ident": ident,
    }


def kernel(q, k, v, Wq, bq, Wk, bk, Wv, bv, Wo, bo):
    from concourse.bass_utils import run_bass_kernel_spmd

    q, k, v = (np.asarray(x, np.float32) for x in (q, k, v))
    Wq, bq, Wk, bk, Wv, bv, Wo, bo = (
        np.asarray(x, np.float32) for x in (Wq, bq, Wk, bk, Wv, bv, Wo, bo))

    if "nc" not in _cache:
        _cache["nc"] = _build()
    nc = _cache["nc"]

    ident = np.eye(128, dtype=np.float32)
    in_maps = [_prep_core_inputs(c, q, k, v, Wq, bq, Wk, bk, Wv, Wo, ident)
               for c in range(NCORES)]
    res = run_bass_kernel_spmd(nc, in_maps, core_ids=list(range(NCORES))).results

    weights = np.empty((B, H, S, S), np.float32)
    out = np.empty((B, S, DIM), np.float32)
    extra = (bv.reshape(DIM) @ Wo + bo).astype(np.float32)  # bv folds via softmax sum=1
    for b in range(B):
        acc = None
        for g in range(HPC):
            r = res[b * HPC + g]
            weights[b, g * HPC:(g + 1) * HPC] = r["w_out"]
            acc = r["o_out"] if acc is None else acc + r["o_out"]
        out[b] = acc + extra
    return out, weights
